# revision 9
# baseline (speedup 1.0000x reference)
# Multi-head attention (B=2, T=4096, DIM=1024, H=16, D=64) with RoPE,
# tensor-parallel over 8 TRN2 NeuronCores: core c handles batch c//4 and
# heads 4*(c%4) .. 4*(c%4)+3. Each core computes its 4 heads end-to-end and
# a partial output projection (row-parallel wo); the host sums the 4
# partials per batch and transposes back.
import math

import numpy as np
import ml_dtypes

B, T, DIM = 2, 4096, 1024
HEADS, HD = 16, 64
N_CORES = 8
HPC = 4          # heads per core
JC = HPC * HD    # 256 projection cols per core
BF16 = ml_dtypes.bfloat16

_PROGRAM = None  # cached program


def _rope_tables_np():
    # matches reference.rope_tables(T, 64) in fp32
    inv_freq = 1.0 / (10000.0 ** (np.arange(0, HD, 2, dtype=np.float32) / HD))
    t = np.arange(T, dtype=np.float32)
    freqs = np.einsum("i,j->ij", t, inv_freq).astype(np.float32)  # [T, 32]
    emb = np.concatenate((freqs, freqs), axis=-1)  # [T, 64]
    cos = np.cos(emb).astype(np.float32)
    sin = np.sin(emb).astype(np.float32)
    # rotate_half: out[d] = q[d]*cos[d] + (-q[d+32] if d<32 else q[d-32])*sin[d]
    sgn = np.where(np.arange(HD) < 32, -1.0, 1.0).astype(np.float32)
    sinS = sin * sgn[None, :]
    return cos, sinS


def _build_program():
    """Build the SPMD Bass program (identical on all 8 cores)."""
    from concourse import bacc
    import concourse.mybir as mybir
    import concourse.tile as tile
    from concourse.masks import make_identity

    BF = mybir.dt.bfloat16
    F32 = mybir.dt.float32
    AF = mybir.ActivationFunctionType

    nc = bacc.Bacc("TRN2", debug=False, num_devices=N_CORES)

    xT = nc.dram_tensor("xT", [DIM, T], BF, kind="ExternalInput")
    wqT = nc.dram_tensor("wqT", [DIM, JC], BF, kind="ExternalInput")
    wkT = nc.dram_tensor("wkT", [DIM, JC], BF, kind="ExternalInput")
    wvT = nc.dram_tensor("wvT", [DIM, JC], BF, kind="ExternalInput")
    wo65 = nc.dram_tensor("wo65", [HD + 1, HPC, DIM], BF, kind="ExternalInput")
    cosn = nc.dram_tensor("cosn", [T, HD], F32, kind="ExternalInput")
    sinn = nc.dram_tensor("sinn", [T, HD], F32, kind="ExternalInput")
    # passthrough input: lets a benchmark chain executions back-to-back
    # (pout of step i fed as chain of step i+1) with no host/XLA transform
    chain = nc.dram_tensor("chain", [DIM, T], F32, kind="ExternalInput")
    pout = nc.dram_tensor("pout", [DIM, T], F32, kind="ExternalOutput")
    chk = nc.dram_tensor("chk", [1, 512], F32, kind="ExternalOutput")

    NCC = DIM // 128     # 8 contraction chunks
    NTB = T // 128       # 32 t-blocks of 128
    NSB = T // 128       # 32 s-blocks of 128
    NTW = T // 512       # 8 t-blocks of 512
    GRP = 16             # QK/exp/PV grouping (PE mode-switch batching)

    with tile.TileContext(nc) as tc:
        with (
            tc.tile_pool(name="const", bufs=1) as constp,
            tc.tile_pool(name="xp", bufs=2) as xp,
            tc.tile_pool(name="ropep", bufs=3) as ropep,
            tc.tile_pool(name="ptp", bufs=10) as ptp,
            tc.tile_pool(name="stagep", bufs=3) as stagep,
            tc.tile_pool(name="normp", bufs=3) as normp,
        ):
            # ---- persistent tiles ----
            ident = constp.tile([128, 128], BF)
            make_identity(nc, ident)

            wq_s = constp.tile([128, NCC, JC], BF)
            nc.sync.dma_start(wq_s, wqT.ap().rearrange("(cc p) j -> p cc j", p=128))
            wk_s = constp.tile([128, NCC, JC], BF)
            nc.sync.dma_start(wk_s, wkT.ap().rearrange("(cc p) j -> p cc j", p=128))
            wv_s = constp.tile([128, NCC, JC], BF)
            nc.sync.dma_start(wv_s, wvT.ap().rearrange("(cc p) j -> p cc j", p=128))
            wo_s = constp.tile([HD + 1, HPC, DIM], BF)
            nc.sync.dma_start(wo_s, wo65.ap())
            cos_s = constp.tile([128, NTB, HD], F32)
            nc.sync.dma_start(cos_s, cosn.ap().rearrange("(tc p) d -> p tc d", p=128))
            sin_s = constp.tile([128, NTB, HD], F32)
            nc.sync.dma_start(sin_s, sinn.ap().rearrange("(tc p) d -> p tc d", p=128))

            zbias = constp.tile([128, 1], F32)
            nc.vector.memset(zbias, 0.0)

            # chain passthrough (negligible: one small DMA in/out)
            chtile = constp.tile([1, 512], F32)
            nc.sync.dma_start(chtile, chain.ap()[0:1, 0:512])
            nc.sync.dma_start(chk.ap(), chtile)

            qTs = constp.tile([128, 2, T], BF)   # [j, t]: j = jb*128+p, head=j//64
            kTs = constp.tile([128, 2, T], BF)
            # v in normal layout per s-chunk; per head: col 0 = ones, 1..64 = d
            v_s = constp.tile([128, NSB, HPC * (HD + 1)], BF)
            yTs = constp.tile([HD + 1, HPC, T], BF)  # row 0 = junk (killed by wo zero row)

            v4 = v_s.rearrange("p sc (h u) -> p sc h u", h=HPC)
            for h in range(HPC):
                nc.vector.memset(v4[:, :, h, 0:1], 1.0)

            # ---- phase 1: projections + RoPE + transpose ----
            with (
                tc.tile_pool(name="psP", bufs=2, space="PSUM") as psP,
                tc.tile_pool(name="psT", bufs=3, space="PSUM") as psT,
            ):
                for tb in range(NTB):
                    xt = xp.tile([128, NCC, 128], BF, tag="xt")
                    nc.sync.dma_start(
                        xt,
                        xT.ap().rearrange("(cc p) t -> p cc t", p=128)[
                            :, :, tb * 128 : (tb + 1) * 128
                        ],
                    )
                    tsl = slice(tb * 128, (tb + 1) * 128)

                    for wt, dstT in ((wq_s, qTs), (wk_s, kTs)):
                        P = psP.tile([128, JC], F32, tag="P")
                        for cc in range(NCC):
                            nc.tensor.matmul(
                                P, lhsT=xt[:, cc, :], rhs=wt[:, cc, :],
                                start=(cc == 0), stop=(cc == NCC - 1),
                            )
                        # RoPE: out = P*cos + swap(P)*sinS  (per 64-wide head)
                        A = ropep.tile([128, JC], F32, tag="A")
                        P4 = P.rearrange("p (h d) -> p h d", h=HPC)
                        ct = (
                            cos_s[:, tb, :]
                            .rearrange("p (o d) -> p o d", o=1)
                            .broadcast_to([128, HPC, HD])
                        )
                        nc.vector.tensor_mul(
                            A.rearrange("p (h d) -> p h d", h=HPC), P4, ct
                        )
                        Bt = ropep.tile([128, JC], F32, tag="B")
                        B4 = Bt.rearrange("p (h u d) -> p h u d", h=HPC, u=2)
                        P42 = P.rearrange("p (h u d) -> p h u d", h=HPC, u=2)
                        s0 = (
                            sin_s[:, tb, 0:32]
                            .rearrange("p (o d) -> p o d", o=1)
                            .broadcast_to([128, HPC, 32])
                        )
                        s1 = (
                            sin_s[:, tb, 32:64]
                            .rearrange("p (o d) -> p o d", o=1)
                            .broadcast_to([128, HPC, 32])
                        )
                        nc.vector.tensor_mul(B4[:, :, 0, :], P42[:, :, 1, :], s0)
                        nc.vector.tensor_mul(B4[:, :, 1, :], P42[:, :, 0, :], s1)
                        qr = ropep.tile([128, JC], BF, tag="qr")
                        nc.vector.tensor_add(qr, A, Bt)
                        for jb in range(2):
                            tp = psT.tile([128, 128], BF, tag="tp")
                            nc.tensor.transpose(
                                tp, qr[:, jb * 128 : (jb + 1) * 128], ident
                            )
                            nc.vector.tensor_copy(dstT[:, jb, tsl], tp)

                    V = psP.tile([128, JC], F32, tag="P")
                    for cc in range(NCC):
                        nc.tensor.matmul(
                            V, lhsT=xt[:, cc, :], rhs=wv_s[:, cc, :],
                            start=(cc == 0), stop=(cc == NCC - 1),
                        )
                    nc.vector.tensor_copy(
                        v4[:, tb, :, 1 : HD + 1],
                        V.rearrange("p (h d) -> p h d", h=HPC),
                    )

            # ---- phases 2+3 psum pools ----
            with (
                tc.tile_pool(name="psS", bufs=2, space="PSUM") as psS,
                tc.tile_pool(name="psO", bufs=2, space="PSUM") as psO,
            ):
                # ---- phase 2: attention. The 64 (sb, head-half) score tiles
                # per (hp, tw) are packed 3-per-psum-tile so each ACT exp
                # instruction covers [128, 1536]; QK runs as K=64 row-tiled
                # matmuls (head A on PE rows 0-63, head B on 64-127). ----
                for hp in range(2):
                    for tw in range(NTW):
                        twsl = slice(tw * 512, (tw + 1) * 512)
                        hA, hB = 2 * hp, 2 * hp + 1
                        oA = psO.tile([HD + 1, 512], F32, tag="o")
                        oB = psO.tile([HD + 1, 512], F32, tag="o")
                        halves = [(sb, hh) for sb in range(NSB) for hh in (0, 1)]
                        triples = [halves[i : i + 3] for i in range(0, len(halves), 3)]
                        SUB = 4  # triples per PE mode-switch batch
                        for bs in range(0, len(triples), SUB):
                            produced = []
                            for tri in triples[bs : bs + SUB]:
                                W = 512 * len(tri)
                                S = psS.tile([128, 1536], F32, tag="s")
                                for kk, (sb, hh) in enumerate(tri):
                                    ssl = slice(sb * 128, (sb + 1) * 128)
                                    nc.tensor.matmul(
                                        S[:, kk * 512 : (kk + 1) * 512],
                                        lhsT=kTs[hh * 64 : (hh + 1) * 64, hp, ssl],
                                        rhs=qTs[hh * 64 : (hh + 1) * 64, hp, twsl],
                                        start=True, stop=True,
                                    )
                                P = ptp.tile([128, 1536], BF, tag="pT")
                                nc.scalar.activation(
                                    P[:, 0:W], S[:, 0:W], AF.Exp,
                                    bias=zbias, scale=0.125,
                                )
                                produced.append((tri, P))
                            for tri, P in produced:
                                for kk, (sb, hh) in enumerate(tri):
                                    o = oA if hh == 0 else oB
                                    hloc = 2 * hp + hh
                                    nc.tensor.matmul(
                                        o,
                                        lhsT=v_s[:, sb, hloc * 65 : hloc * 65 + 65],
                                        rhs=P[:, kk * 512 : (kk + 1) * 512],
                                        start=(sb == 0), stop=(sb == NSB - 1),
                                    )
                        for o, h in ((oA, hA), (oB, hB)):
                            rc = normp.tile([1, 512], F32, tag="rc")
                            nc.vector.reciprocal(rc, o[0:1, :])
                            bc = normp.tile([HD + 1, 512], F32, tag="bc")
                            nc.gpsimd.partition_broadcast(bc, rc)
                            nc.vector.tensor_mul(yTs[:, h, twsl], o, bc)

                # ---- phase 3: output projection (K=65; zero wo row kills
                # the denominator row) ----
                for cb in range(8):
                    cbsl = slice(cb * 128, (cb + 1) * 128)
                    for tw in range(NTW):
                        twsl = slice(tw * 512, (tw + 1) * 512)
                        po = psS.tile([128, 512], F32, tag="s")
                        for h in range(HPC):
                            nc.tensor.matmul(
                                po, lhsT=wo_s[:, h, cbsl], rhs=yTs[:, h, twsl],
                                start=(h == 0), stop=(h == HPC - 1),
                            )
                        st = stagep.tile([128, 512], F32, tag="st")
                        nc.vector.tensor_copy(st, po)
                        nc.sync.dma_start(pout.ap()[cbsl, twsl], st)

    nc.compile()
    return nc


def _get_program():
    global _PROGRAM
    if _PROGRAM is None:
        _PROGRAM = _build_program()
    return _PROGRAM


def make_in_maps(x, wq, wk, wv, wo):
    """Host-side sharding/layout prep: per-core input dicts."""
    x = np.asarray(x, dtype=np.float32)
    wq = np.asarray(wq, dtype=np.float32)
    wk = np.asarray(wk, dtype=np.float32)
    wv = np.asarray(wv, dtype=np.float32)
    wo = np.asarray(wo, dtype=np.float32)
    cos, sinS = _rope_tables_np()

    xT_b = [np.ascontiguousarray(x[b].T).astype(BF16) for b in range(B)]
    in_maps = []
    for c in range(N_CORES):
        b, hg = divmod(c, HPC)
        jsl = slice(hg * JC, (hg + 1) * JC)
        wqTc = np.ascontiguousarray(wq[jsl, :].T).astype(BF16)
        wkTc = np.ascontiguousarray(wk[jsl, :].T).astype(BF16)
        wvTc = np.ascontiguousarray(wv[jsl, :].T).astype(BF16)
        # wo65[0] = 0; wo65[1+d, h, co] = wo[co, hg*256 + h*64 + d]
        wo65 = np.zeros((HD + 1, HPC, DIM), dtype=np.float32)
        wo_cols = wo[:, jsl]  # [DIM, 256]
        wo65[1:, :, :] = wo_cols.reshape(DIM, HPC, HD).transpose(2, 1, 0)
        in_maps.append(
            {
                "xT": xT_b[b],
                "wqT": wqTc,
                "wkT": wkTc,
                "wvT": wvTc,
                "wo65": wo65.astype(BF16),
                "cosn": cos,
                "sinn": sinS,
                "chain": _ZCHAIN,
            }
        )
    return in_maps


_ZCHAIN = np.zeros((DIM, T), dtype=np.float32)


def assemble(results):
    """Host-side unshard: sum 4 head-group partials per batch, transpose."""
    out = np.zeros((B, T, DIM), dtype=np.float32)
    for b in range(B):
        acc = np.zeros((DIM, T), dtype=np.float32)
        for hg in range(HPC):
            acc += results[b * HPC + hg]["pout"]
        out[b] = acc.T
    return out


def kernel(x, wq, wk, wv, wo):
    from concourse.bass_utils import run_bass_kernel_spmd

    nc = _get_program()
    in_maps = make_in_maps(x, wq, wk, wv, wo)
    res = run_bass_kernel_spmd(nc, in_maps, core_ids=list(range(N_CORES)))
    return assemble(res.results)


if __name__ == "__main__":
    nc = _get_program()
    print("program built + compiled OK")
